# revision 3
# baseline (speedup 1.0000x reference)
"""Trainium2 Bass kernel for nn_CustomLoss_30940944401001.

loss = sum_j relu(1 - (P[s_j, a_j] - P[s_j, b_j])) / B,  P = pred @ Xi

Strategy (8 NeuronCores, data-parallel over batch rows):
  - Each core owns 1024 rows of pred; Xi replicated. P computed on-device as
    bf16 matmul (8 m-tiles x 8 psum-chunks x 8 k-tiles).
  - The per-item gather P[s,a]/P[s,b] is inverted on host into per-partition
    scatters: one GPSIMD local_scatter per m-tile routes each needed P column
    value into item-paired slots (A half / B half of a slot region).
  - A (row, col) value can be scattered to only one slot per pass, so items
    that lose a column claim ("overflow", ~3%) are served by tiny chain
    scatter passes that copy from the winner's slot in the pass-1 output
    (scan length = slot region, not P).
  - Hinge: DVE subtract (B-half - A-half), ACT relu(x+1) with fused
    per-partition accumulation. Host sums partials, subtracts the exact
    count of padding slots (each contributes exactly 1.0) and adds the
    a==b items (hinge exactly 1.0), then divides by B.
"""

import os
import sys
import types

import numpy as np
import ml_dtypes


# ----------------------------------------------------------------------------
# Environment shims (axon NTFF profile hook + walrus workarounds)
# ----------------------------------------------------------------------------

def _ensure_axon_profile_hook():
    """The image's antenv package lacks axon_hooks; provide it so
    run_bass_kernel_spmd(trace=True) can profile via the axon .so."""
    if "antenv.axon_hooks" in sys.modules:
        return
    try:
        from trn_agent_boot.trn_boot import _ntff_profile_via_ctypes
    except Exception:
        return
    mod = types.ModuleType("antenv.axon_hooks")
    hook = [None]
    mod.set_axon_ntff_profile_hook = lambda h: hook.__setitem__(0, h)
    mod.get_axon_ntff_profile_hook = lambda: hook[0]
    sys.modules["antenv.axon_hooks"] = mod
    try:
        mod.set_axon_ntff_profile_hook(
            _ntff_profile_via_ctypes("/opt/axon/libaxon_pjrt.so")
        )
    except Exception:
        pass


_ensure_axon_profile_hook()

import concourse.bass as bass  # noqa: E402
import concourse.tile as tile  # noqa: E402
import concourse.mybir as mybir  # noqa: E402
from concourse.bass_utils import run_bass_kernel_spmd  # noqa: E402


def _patched_drain_and_barrier(self, tick_clock, wait_clock):
    from concourse.vector_clock import ScopedClock

    nc_ = self.nc
    drain_inst = nc_.sync.drain()
    wait_clock.add_sem_waits(
        drain_inst.ins, ScopedClock({None: tick_clock.global_clock})
    )
    si = drain_inst.ins.sync_info
    if si is not None and si.on_wait and len(si.on_wait) > 1:
        waits = list(si.on_wait)
        drain_inst.ins.sync_info = mybir.SyncInfo(
            on_wait=[waits[0]], on_update=si.on_update
        )
        for w in waits[1:]:
            d2 = nc_.sync.drain()
            d2.ins.sync_info = mybir.SyncInfo(on_wait=[w], on_update=[])
    nc_.all_engine_barrier()
    assert self.sems is not None
    popped = nc_._tile_sem_poison_stack.pop()
    assert popped is self._sem_poison
    nc_.clear_and_free_semaphores(list(self.sems.allocated().values()))
    nc_.all_engine_barrier()


tile.TileContext._drain_and_barrier = _patched_drain_and_barrier


def _split_multi_waits(nc):
    """This walrus build allows one sync-wait command per instruction; split
    any instruction with N>1 waits into N-1 single-wait NOPs ahead of it."""
    ctr = [0]

    def mknop(engine, wait):
        ctr[0] += 1
        return mybir.InstNoOp(
            name=f"waitsplit-{ctr[0]}",
            engine=engine,
            ins=[],
            outs=[],
            sync_info=mybir.SyncInfo(on_wait=[wait], on_update=[]),
        )

    for f in nc.m.functions:
        for blk in f.blocks:
            insts = blk.instructions
            out = []
            changed = False
            for inst in insts:
                si = inst.sync_info
                if si is not None and si.on_wait and len(si.on_wait) > 1:
                    waits = list(si.on_wait)
                    for w in waits[:-1]:
                        out.append(mknop(inst.engine, w))
                    inst.sync_info = mybir.SyncInfo(
                        on_wait=[waits[-1]], on_update=si.on_update
                    )
                    changed = True
                out.append(inst)
            if changed:
                blk.instructions = out


def _lower_libraries(nc):
    """Raw Bass skips Bacc's library lowering: insert GPSIMD library loads
    and fill the ISA bytes of extended instructions."""
    import bass_rust
    from concourse.library_config import all_libraries, standard

    inst_type_to_lib_mask = {}
    for lib in all_libraries:
        for inst_type in lib.instructions:
            inst_type_to_lib_mask[inst_type] = inst_type_to_lib_mask.get(
                inst_type, 0
            ) | (1 << lib.index)
    bass_rust.insert_library_loads(
        nc, inst_type_to_lib_mask, len(all_libraries), standard.index
    )
    mybir.codegen_inst_isa_subclasses(nc)


def _finalize(nc, for_sim=False):
    if not for_sim:
        _split_multi_waits(nc)  # breaks CoreSim's event loop; HW-only
    _lower_libraries(nc)


# ----------------------------------------------------------------------------
# Problem configuration
# ----------------------------------------------------------------------------

class Cfg:
    def __init__(self, B=8192, D=1024, K=4096, n_cores=8):
        self.B, self.D, self.K, self.n_cores = B, D, K, n_cores
        self.R = B // n_cores          # rows per core
        self.MT = self.R // 128        # m-tiles per core
        self.KT = D // 128             # k-tiles (contraction)
        self.NCH = K // 512            # psum chunks
        assert self.R % 128 == 0 and D % 128 == 0 and K % 512 == 0


FULL = Cfg()


# ----------------------------------------------------------------------------
# Host planning: invert the gather into scatter index arrays
# ----------------------------------------------------------------------------

def _even(x):
    return int(x) + (int(x) & 1)


def _plan(seg, a, b, cfg):
    """Build per-core scatter/chain index arrays and exact corrections.

    Returns dict with:
      scat:   [n_cores, MT, 128, K] int16   pass-1 scatter idx
      chains: list of [n_cores, 128, MT*S1] int16
      sizes:  PAIRS, SRC, S1, CH (list)
      pads:   per-core float correction counts
      n_self: count of a==b items
    """
    B, K, n_cores, MT = cfg.B, cfg.K, cfg.n_cores, cfg.MT
    seg = np.asarray(seg, dtype=np.int64)
    a = np.asarray(a, dtype=np.int64)
    b = np.asarray(b, dtype=np.int64)

    selfpair = a == b
    n_self = int(selfpair.sum())
    keep = ~selfpair
    seg_k, a_k, b_k = seg[keep], a[keep], b[keep]
    n = len(seg_k)

    # side arrays: 2n entries, [A sides | B sides]
    side_row = np.concatenate([seg_k, seg_k])
    side_col = np.concatenate([a_k, b_k])
    key = side_row * K + side_col

    order = np.argsort(key, kind="stable")
    skey = key[order]
    is_start = np.r_[True, skey[1:] != skey[:-1]]
    gid = np.cumsum(is_start) - 1
    pos = np.arange(2 * n)
    gstart = pos[is_start][gid]
    rank_sorted = pos - gstart
    rank = np.empty(2 * n, np.int64)
    rank[order] = rank_sorted
    rank_a, rank_b = rank[:n], rank[n:]

    main = (rank_a == 0) & (rank_b == 0)
    over = ~main

    # --- main item slot assignment: q = cumcount within row ---
    rows_m = seg_k[main]
    ord_m = np.argsort(rows_m, kind="stable")
    rm_sorted = rows_m[ord_m]
    st = np.r_[True, rm_sorted[1:] != rm_sorted[:-1]]
    gi = np.cumsum(st) - 1
    p2 = np.arange(len(rm_sorted))
    q_sorted = p2 - p2[st][gi]
    q = np.empty(len(rows_m), np.int64)
    q[ord_m] = q_sorted
    n_main_per_row = np.bincount(rows_m, minlength=B)
    PAIRS = _even(n_main_per_row.max() if len(rows_m) else 0)

    # --- source-only slots -------------------------------------------------
    # keys whose rank-0 side belongs to an overflow item have no main-item
    # slot; allocate a source slot so chain passes can copy from pass-1 out.
    # A key needs a source slot iff some overflow item references it and its
    # rank-0 side is not from a main item.
    main_side = np.concatenate([main, main])  # side belongs to a main item
    # winner side (rank 0) per group, in sorted order:
    win_sorted_pos = pos[is_start]            # positions in sorted order
    win_side = order[win_sorted_pos]          # side index of each group winner
    win_is_main = main_side[win_side]
    # groups referenced by overflow items:
    side_over = np.concatenate([over, over])
    grp_of_side = np.empty(2 * n, np.int64)
    grp_of_side[order] = gid
    over_groups = np.unique(grp_of_side[side_over])
    need_src_groups = over_groups[~win_is_main[over_groups]]
    # allocate source slot per such group, indexed per row
    src_rows = side_row[win_side[need_src_groups]]
    src_cols = side_col[win_side[need_src_groups]]
    ord_s = np.argsort(src_rows, kind="stable")
    sr_sorted = src_rows[ord_s]
    st = np.r_[True, sr_sorted[1:] != sr_sorted[:-1]] if len(sr_sorted) else np.array([], bool)
    if len(sr_sorted):
        gi = np.cumsum(st) - 1
        p3 = np.arange(len(sr_sorted))
        s_sorted = p3 - p3[st][gi]
        s_idx = np.empty(len(src_rows), np.int64)
        s_idx[ord_s] = s_sorted
        n_src_per_row = np.bincount(src_rows, minlength=B)
        SRC = int(n_src_per_row.max())
    else:
        s_idx = np.zeros(0, np.int64)
        SRC = 0
    S1 = _even(2 * PAIRS + SRC)
    assert S1 < 2047, S1

    # --- pass-1 scatter array ---------------------------------------------
    # scat[core, t, p, col]: slot in the (row, t) region or -1
    scat = np.full((n_cores, MT, 128, K), -1, dtype=np.int16)
    scat_flat = scat.reshape(-1)

    def flatpos(row, col):
        core = row // (MT * 128)
        local = row % (MT * 128)
        t = local // 128
        p = local % 128
        return ((core * MT + t) * 128 + p) * K + col

    # main item A sides -> slot q, B sides -> slot PAIRS + q
    scat_flat[flatpos(rows_m, a_k[main])] = q
    scat_flat[flatpos(rows_m, b_k[main])] = PAIRS + q
    # source-only cols -> slot 2*PAIRS + s
    if len(src_rows):
        scat_flat[flatpos(src_rows, src_cols)] = 2 * PAIRS + s_idx

    # --- L1 position of each key's value (for chain passes) ----------------
    # slot within row region (int) for each group that has a value in L1
    n_groups = int(is_start.sum())
    grp_slot = np.full(max(n_groups, 1), -1, np.int64)
    # main sides: group of side -> slot
    slot_of_side = np.full(2 * n, -1, np.int64)
    item_slotA = np.full(n, -1, np.int64)
    item_slotB = np.full(n, -1, np.int64)
    item_slotA[main] = q
    item_slotB[main] = PAIRS + q
    slot_of_side[:n] = item_slotA
    slot_of_side[n:] = item_slotB
    ok = win_is_main
    grp_slot[np.arange(len(win_side))[ok]] = slot_of_side[win_side[ok]]
    if len(need_src_groups):
        grp_slot[need_src_groups] = 2 * PAIRS + s_idx
    assert (grp_slot[over_groups] >= 0).all()

    # --- chain pass planning (edge coloring) -------------------------------
    ov_items = np.nonzero(over)[0]
    ov_row = seg_k[ov_items]
    ov_ga = grp_of_side[ov_items]          # A-side group
    ov_gb = grp_of_side[n + ov_items]      # B-side group
    # greedy edge coloring: smallest color free at both keys
    used = {}
    colors = np.zeros(len(ov_items), np.int64)
    for i in range(len(ov_items)):
        ga, gb = int(ov_ga[i]), int(ov_gb[i])
        ua = used.setdefault(ga, set())
        ub = used.setdefault(gb, set())
        c = 0
        while c in ua or c in ub:
            c += 1
        ua.add(c)
        ub.add(c)
        colors[i] = c
    NC = int(colors.max()) + 1 if len(ov_items) else 0

    # positions in pass-1 output (flat [MT*S1] per partition)
    ov_t = (ov_row % (MT * 128)) // 128
    ov_p = ov_row % 128
    ov_core = ov_row // (MT * 128)
    srcA = ov_t * S1 + grp_slot[ov_ga]
    srcB = ov_t * S1 + grp_slot[ov_gb]

    chains = []
    CH = []
    for c in range(NC):
        m = colors == c
        # r = cumcount within (core, p) for this color
        cp = ov_core[m] * 128 + ov_p[m]
        ord_c = np.argsort(cp, kind="stable")
        cps = cp[ord_c]
        st = np.r_[True, cps[1:] != cps[:-1]] if len(cps) else np.array([], bool)
        if len(cps):
            gi = np.cumsum(st) - 1
            pz = np.arange(len(cps))
            r_sorted = pz - pz[st][gi]
            r = np.empty(len(cps), np.int64)
            r[ord_c] = r_sorted
            n_per_cp = np.bincount(cp, minlength=n_cores * 128)
            half = int(n_per_cp.max())
        else:
            r = np.zeros(0, np.int64)
            half = 0
        half = max(half, 1)
        ch_sz = _even(2 * half)
        CH.append(ch_sz)
        arr = np.full((n_cores, 128, MT * S1), -1, dtype=np.int16)
        af = arr.reshape(-1)
        base = (ov_core[m] * 128 + ov_p[m]) * (MT * S1)
        af[base + srcA[m]] = r
        af[base + srcB[m]] = half + r
        chains.append(arr)
    # pad counts
    pads = np.zeros(n_cores, np.float64)
    # main region pads: sum over rows of (PAIRS - n_main_per_row)
    row_core = np.arange(B) // (MT * 128)
    pads += np.bincount(row_core, weights=(PAIRS - n_main_per_row), minlength=n_cores)
    for c in range(NC):
        m = colors == c
        half = CH[c] // 2
        n_per_cp = np.bincount(ov_core[m] * 128 + ov_p[m], minlength=n_cores * 128)
        n_per_cp = n_per_cp.reshape(n_cores, 128)
        pads += (half - n_per_cp).sum(axis=1)

    # sanity: every non-self item accounted once
    assert int(main.sum()) + len(ov_items) == n

    return dict(
        scat=scat,
        chains=chains,
        PAIRS=PAIRS,
        SRC=SRC,
        S1=S1,
        CH=CH,
        NC=NC,
        pads=pads,
        n_self=n_self,
    )


# ----------------------------------------------------------------------------
# Bass program
# ----------------------------------------------------------------------------

def _build(cfg, S1, PAIRS, CH, for_sim=False):
    MT, KT, NCH, K, R = cfg.MT, cfg.KT, cfg.NCH, cfg.K, cfg.R
    NC = len(CH)
    n_pcols = MT + NC

    nc = bass.Bass("TRN2", target_bir_lowering=False, debug=False)
    predT_d = nc.dram_tensor("predT", [128, KT * R], mybir.dt.bfloat16, kind="ExternalInput")
    xi_d = nc.dram_tensor("xi", [128, KT * K], mybir.dt.bfloat16, kind="ExternalInput")
    scat_d = nc.dram_tensor("scat", [MT, 128, K], mybir.dt.int16, kind="ExternalInput")
    chain_ds = [
        nc.dram_tensor(f"chain{c}", [128, MT * S1], mybir.dt.int16, kind="ExternalInput")
        for c in range(NC)
    ]
    out_d = nc.dram_tensor("partials", [128, n_pcols], mybir.dt.float32, kind="ExternalOutput")

    Relu = mybir.ActivationFunctionType.Relu

    with tile.TileContext(nc) as tc:
        with (
            tc.tile_pool(name="weights", bufs=1) as wpool,
            tc.tile_pool(name="scatidx", bufs=2) as spool,
            tc.tile_pool(name="ptile", bufs=3) as ppool,
            tc.tile_pool(name="hinge", bufs=2) as hpool,
            tc.tile_pool(name="acc", bufs=4, space="PSUM") as psum_pool,
            tc.tile_pool(name="misc", bufs=1) as mpool,
        ):
            predT_t = wpool.tile([128, KT, R], mybir.dt.bfloat16)
            xi_t = wpool.tile([128, KT, K], mybir.dt.bfloat16)
            nc.sync.dma_start(predT_t[:], predT_d.ap().rearrange("p (kt r) -> p kt r", kt=KT))
            nc.sync.dma_start(xi_t[:], xi_d.ap().rearrange("p (kt k) -> p kt k", kt=KT))

            dst_all = mpool.tile([128, MT, S1], mybir.dt.bfloat16)
            partials = mpool.tile([128, n_pcols], mybir.dt.float32)

            for t in range(MT):
                idx_t = spool.tile([128, K], mybir.dt.int16, tag="scat")
                nc.sync.dma_start(idx_t[:], scat_d[t, :, :])

                p_sb = ppool.tile([128, K], mybir.dt.bfloat16, tag="p")
                for nn in range(NCH):
                    acc = psum_pool.tile([128, 512], mybir.dt.float32, tag="acc")
                    for kt in range(KT):
                        nc.tensor.matmul(
                            acc[:],
                            predT_t[:, kt, t * 128:(t + 1) * 128],
                            xi_t[:, kt, nn * 512:(nn + 1) * 512],
                            start=(kt == 0),
                            stop=(kt == KT - 1),
                        )
                    nc.scalar.copy(p_sb[:, nn * 512:(nn + 1) * 512], acc[:])

                nc.gpsimd.local_scatter(
                    dst_all[:, t, :], p_sb[:], idx_t[:],
                    channels=128, num_elems=S1, num_idxs=K,
                )

                diff = hpool.tile([128, PAIRS], mybir.dt.float32, tag="diff")
                nc.vector.tensor_sub(
                    diff[:], dst_all[:, t, PAIRS:2 * PAIRS], dst_all[:, t, 0:PAIRS]
                )
                relu_o = hpool.tile([128, PAIRS], mybir.dt.float32, tag="relu")
                nc.scalar.activation(
                    relu_o[:], diff[:], Relu, bias=1.0, scale=1.0,
                    accum_out=partials[:, t:t + 1],
                )

            data_flat = dst_all[:].rearrange("p t s -> p (t s)")
            for c in range(NC):
                cidx = spool.tile([128, MT * S1], mybir.dt.int16, tag="chain")
                nc.sync.dma_start(cidx[:], chain_ds[c][:])
                dstc = hpool.tile([128, CH[c]], mybir.dt.bfloat16, tag=f"dstc{c}")
                nc.gpsimd.local_scatter(
                    dstc[:], data_flat, cidx[:],
                    channels=128, num_elems=CH[c], num_idxs=MT * S1,
                )
                half = CH[c] // 2
                diffc = hpool.tile([128, half], mybir.dt.float32, tag=f"diffc{c}")
                nc.vector.tensor_sub(diffc[:], dstc[:, half:], dstc[:, 0:half])
                reluc = hpool.tile([128, half], mybir.dt.float32, tag=f"reluc{c}")
                nc.scalar.activation(
                    reluc[:], diffc[:], Relu, bias=1.0, scale=1.0,
                    accum_out=partials[:, MT + c:MT + c + 1],
                )

            nc.sync.dma_start(out_d[:], partials[:])

    _finalize(nc, for_sim=for_sim)
    return nc


# ----------------------------------------------------------------------------
# Host data prep
# ----------------------------------------------------------------------------

def _prep_core_inputs(pred, Xi, plan, cfg):
    MT, KT, K, R, n_cores = cfg.MT, cfg.KT, cfg.K, cfg.R, cfg.n_cores
    D = cfg.D
    Xi_b = np.ascontiguousarray(Xi.astype(ml_dtypes.bfloat16))
    # xi sbuf layout: [128, KT, K]; value at (p, kt, k) = Xi[kt*128+p, k]
    xi_sb = np.ascontiguousarray(
        Xi_b.reshape(KT, 128, K).transpose(1, 0, 2)
    ).reshape(128, KT * K)

    in_maps = []
    for c in range(n_cores):
        pc = pred[c * R:(c + 1) * R, :].astype(ml_dtypes.bfloat16)
        # predT sbuf layout: [128, KT, R]; value (p, kt, r) = pred[r, kt*128+p]
        predT_sb = np.ascontiguousarray(
            pc.T.reshape(KT, 128, R).transpose(1, 0, 2)
        ).reshape(128, KT * R)
        m = {
            "predT": predT_sb,
            "xi": xi_sb,
            "scat": np.ascontiguousarray(plan["scat"][c]),
        }
        for ci, arr in enumerate(plan["chains"]):
            m[f"chain{ci}"] = np.ascontiguousarray(arr[c])
        in_maps.append(m)
    return in_maps


def _assemble(results, plan, cfg):
    total = 0.0
    for c in range(cfg.n_cores):
        total += float(np.asarray(results[c]["partials"], dtype=np.float64).sum())
    total -= float(plan["pads"].sum())
    total += float(plan["n_self"])
    return np.float32(total / cfg.B)


_LAST_EXEC_NS = None


def kernel(pred, Xi, yhat_idx, ytilde_idx, seg_ids):
    global _LAST_EXEC_NS
    cfg = FULL
    assert pred.shape == (cfg.B, cfg.D) and Xi.shape == (cfg.D, cfg.K)

    plan = _plan(seg_ids, yhat_idx, ytilde_idx, cfg)
    nc = _build(cfg, plan["S1"], plan["PAIRS"], plan["CH"])
    in_maps = _prep_core_inputs(np.asarray(pred), np.asarray(Xi), plan, cfg)

    trace = os.environ.get("KERNEL_TRACE", "0") == "1"
    last_err = None
    for _attempt in range(3):
        try:
            res = run_bass_kernel_spmd(
                nc, in_maps, core_ids=list(range(cfg.n_cores)), trace=trace
            )
            break
        except Exception as e:  # transient device errors happen; retry
            last_err = e
    else:
        raise last_err
    _LAST_EXEC_NS = res.exec_time_ns
    return _assemble(res.results, plan, cfg)
